# revision 8
# baseline (speedup 1.0000x reference)
"""Cross-attention (B=4, NQ=1024, P=2048, D=1024, H=16) on 8 trn2 NeuronCores.

Sharding: data-parallel over batch (4) x query-rows (2): core c handles
batch c//2, query rows (c%2)*512:(c%2)*512+512.  Each core runs the full
pipeline locally (K/V projections are duplicated within a batch pair), so
no collectives are needed and LayerNorm is fully local.

Device-side layout notes:
  * All matmul operands use the f32r dtype (full-rate fp32 matmul on the
    PE when the moving free dim >= 256; ~1e-4 relative rounding).
  * All host->device tensors are pre-transposed on the host so every DMA
    is a contiguous row load (contraction dim on partitions).
  * Attention computes S^T = (K_h Q_h^T) with keys on partitions, so the
    softmax denominator is obtained with an ones-column appended to V
    (O_aug = [V | 1]^T P) instead of a partition-axis reduction, and the
    exp() never needs a running max (scores are O(1) for these inputs:
    |S|max ~ 5, exp range is safe in fp32).
"""

import os
import sys

for _p in ("/opt/trn_rl_repo", "/root/.axon_site/_ro/trn_rl_repo"):
    if os.path.isdir(_p) and _p not in sys.path:
        sys.path.insert(0, _p)

import numpy as np

import concourse.bass as bass
import concourse.mybir as mybir
import concourse.tile as tile
from concourse import bacc
from concourse.bass_utils import run_bass_kernel_spmd

F32 = mybir.dt.float32
F32R = mybir.dt.float32r
AF = mybir.ActivationFunctionType
OP = mybir.AluOpType

B, NQ, P, D, H, DK = 4, 1024, 2048, 1024, 16, 64
EPS = 1e-5
NQS = NQ // 2          # query rows per core
NT = D // 128          # 8 tiles over D
NKT = P // 128         # 16 tiles over keys
NPASS = 4              # head-quarter passes
HPP = H // NPASS       # 4 heads per pass
SCALE = 1.0 / np.sqrt(DK)


def _bcast(ap, parts=128):
    """DRAM 1-D tensor -> [parts, n] broadcast AP (partition step 0)."""
    return bass.AP(tensor=ap.tensor, offset=ap.offset, ap=[[0, parts]] + list(ap.ap))


def _build():
    nc = bacc.Bacc(None, target_bir_lowering=False)

    qT = nc.dram_tensor("qT", [D, NQS], F32R, kind="ExternalInput")
    CT = nc.dram_tensor("CT", [D, P], F32R, kind="ExternalInput")
    WqT = nc.dram_tensor("WqT", [D, D], F32R, kind="ExternalInput")
    WkT = nc.dram_tensor("WkT", [D, D], F32R, kind="ExternalInput")
    WvT = nc.dram_tensor("WvT", [D, D], F32R, kind="ExternalInput")
    WoT = nc.dram_tensor("WoT", [D, D], F32R, kind="ExternalInput")
    bq = nc.dram_tensor("bq", [D], F32, kind="ExternalInput")
    bk = nc.dram_tensor("bk", [D], F32, kind="ExternalInput")
    bv = nc.dram_tensor("bv", [D], F32, kind="ExternalInput")
    bo = nc.dram_tensor("bo", [D], F32, kind="ExternalInput")
    lnw = nc.dram_tensor("lnw", [D], F32, kind="ExternalInput")
    lnb = nc.dram_tensor("lnb", [D], F32, kind="ExternalInput")
    ones64 = nc.dram_tensor("ones64", [DK], F32R, kind="ExternalInput")
    out = nc.dram_tensor("out", [NQS, D], F32, kind="ExternalOutput")

    with tile.TileContext(nc) as tc:
        with (
            tc.tile_pool(name="const", bufs=1) as const,
            tc.tile_pool(name="big", bufs=1) as big,
            tc.tile_pool(name="wk", bufs=2) as wkp,
            tc.tile_pool(name="wv", bufs=2) as wvp,
            tc.tile_pool(name="cts", bufs=2) as ctp,
            tc.tile_pool(name="pt", bufs=2) as ptp,
            tc.tile_pool(name="yo", bufs=2) as yop,
            tc.tile_pool(name="misc", bufs=4) as misc,
            tc.tile_pool(name="ps", bufs=2, space="PSUM") as psp,
            tc.tile_pool(name="sps", bufs=2, space="PSUM") as spsp,
            tc.tile_pool(name="oa", bufs=2, space="PSUM") as oap,
            tc.tile_pool(name="bc", bufs=1, space="PSUM") as bcp,
        ):
            # ---- constants -------------------------------------------------
            bvb = const.tile([128, D], F32, tag="bvb")
            bob = const.tile([128, D], F32, tag="bob")
            lnwb = const.tile([128, D], F32, tag="lnwb")
            lnbb = const.tile([128, D], F32, tag="lnbb")
            nc.gpsimd.dma_start(out=bvb, in_=_bcast(bv[:]))
            nc.gpsimd.dma_start(out=bob, in_=_bcast(bo[:]))
            nc.gpsimd.dma_start(out=lnwb, in_=_bcast(lnw[:]))
            nc.gpsimd.dma_start(out=lnbb, in_=_bcast(lnb[:]))
            bqc = const.tile([128, NT], F32, tag="bqc")
            bkc = const.tile([128, NT], F32, tag="bkc")
            nc.sync.dma_start(out=bqc, in_=bq[:].rearrange("(t p) -> p t", p=128))
            nc.sync.dma_start(out=bkc, in_=bk[:].rearrange("(t p) -> p t", p=128))
            eps_sb = const.tile([128, 1], F32, tag="eps")
            nc.vector.memset(eps_sb, EPS)
            ones_sb = const.tile([1, DK], F32R, tag="ones")
            nc.sync.dma_start(out=ones_sb, in_=ones64[None, :])

            # ---- persistent activations -----------------------------------
            QT_sb = big.tile([128, NT, NQS], F32R, tag="qt")    # Q^T, all heads
            OT_sb = big.tile([128, NT, NQS], F32R, tag="ot")    # O^T, all heads

            # ---- Q projection: Q^T[do, nq] = Wq @ q^T + bq ----------------
            qTs = big.tile([128, NT, NQS], F32R, tag="qin")
            nc.sync.dma_start(out=qTs, in_=qT[:, :].rearrange("(t p) n -> p t n", p=128))
            for c in range(4):  # 256-wide chunks of do
                wq = wkp.tile([128, NT, 256], F32R, tag="wk")
                nc.sync.dma_start(
                    out=wq,
                    in_=WqT[:, c * 256:(c + 1) * 256].rearrange("(t p) n -> p t n", p=128),
                )
                for t2 in range(2):
                    t = c * 2 + t2
                    ps = psp.tile([128, NQS], F32, tag="ps")
                    for dt in range(NT):
                        nc.tensor.matmul(
                            ps,
                            wq[:, dt, t2 * 128:(t2 + 1) * 128],
                            qTs[:, dt, :],
                            start=(dt == 0),
                            stop=(dt == NT - 1),
                        )
                    nc.vector.tensor_scalar_add(QT_sb[:, t, :], ps, bqc[:, t:t + 1])

            # ---- per-pass: K^T/V projections + attention ------------------
            for X in range(NPASS):
                hb = X * HPP * DK          # do-column base of this head group
                KT_sb = big.tile([128, 2, P], F32R, tag="kt")   # K^T, 2 do-tiles
                Vaug = big.tile([128, NKT, HPP, DK + 1], F32R, tag="vaug")
                nc.gpsimd.dma_start(out=Vaug[:, :, :, DK:DK + 1], in_=_bcast(ones64[:]))

                wk = wkp.tile([128, NT, 256], F32R, tag="wk")
                nc.sync.dma_start(
                    out=wk,
                    in_=WkT[:, hb:hb + 256].rearrange("(t p) n -> p t n", p=128),
                )
                wv = wvp.tile([128, NT, 256], F32R, tag="wv")
                nc.sync.dma_start(
                    out=wv,
                    in_=WvT[:, hb:hb + 256].rearrange("(t p) n -> p t n", p=128),
                )

                for pc in range(P // 256):
                    cts = ctp.tile([128, NT, 256], F32R, tag="cts")
                    nc.sync.dma_start(
                        out=cts,
                        in_=CT[:, pc * 256:(pc + 1) * 256].rearrange(
                            "(t p) n -> p t n", p=128),
                    )
                    for t2 in range(2):     # K^T do-tiles of this pass
                        ps = psp.tile([128, 256], F32, tag="ps")
                        for dt in range(NT):
                            nc.tensor.matmul(
                                ps,
                                wk[:, dt, t2 * 128:(t2 + 1) * 128],
                                cts[:, dt, :],
                                start=(dt == 0),
                                stop=(dt == NT - 1),
                            )
                        tglob = X * 2 + t2
                        nc.vector.tensor_scalar_add(
                            KT_sb[:, t2, pc * 256:(pc + 1) * 256], ps,
                            bkc[:, tglob:tglob + 1])
                    for kt2 in range(2):    # V k-tiles within this chunk
                        kt = pc * 2 + kt2
                        ps = psp.tile([128, 256], F32, tag="ps")
                        for dt in range(NT):
                            nc.tensor.matmul(
                                ps,
                                cts[:, dt, kt2 * 128:(kt2 + 1) * 128],
                                wv[:, dt, :],
                                start=(dt == 0),
                                stop=(dt == NT - 1),
                            )
                        nc.vector.tensor_add(
                            Vaug[:, kt, :, 0:DK],
                            ps.rearrange("p (h d) -> p h d", h=HPP),
                            bvb[:, hb:hb + 256].rearrange("p (h d) -> p h d", h=HPP),
                        )

                # ---- attention for this pass's heads ----------------------
                for hh in range(HPP):
                    h = X * HPP + hh
                    tloc, prow = hh // 2, (hh % 2) * DK
                    tq, qrow = h // 2, (h % 2) * DK
                    oa = oap.tile([DK + 1, NQS], F32, tag="oa")
                    for ch in range(4):          # 4 k-tiles per exp chunk
                        pt = ptp.tile([128, 4, NQS], F32R, tag="pt")
                        for j in range(4):
                            kt = ch * 4 + j
                            sps = spsp.tile([128, NQS], F32, tag="sps")
                            nc.tensor.matmul(
                                sps,
                                KT_sb[prow:prow + DK, tloc, kt * 128:(kt + 1) * 128],
                                QT_sb[qrow:qrow + DK, tq, :],
                                start=True, stop=True,
                            )
                            nc.vector.tensor_copy(pt[:, j, :], sps)
                        nc.scalar.activation(pt[:, :, :], pt[:, :, :], AF.Exp,
                                             scale=float(SCALE))
                        for j in range(4):
                            kt = ch * 4 + j
                            nc.tensor.matmul(
                                oa,
                                Vaug[:, kt, hh, :],
                                pt[:, j, :],
                                start=(kt == 0),
                                stop=(kt == NKT - 1),
                            )
                    rc = misc.tile([1, NQS], F32R, tag="rc")
                    with nc.allow_low_precision(reason="f32r keeps ~19 mantissa bits"):
                        nc.vector.reciprocal(rc, oa[DK:DK + 1, :])
                    bc = bcp.tile([DK, NQS], F32, tag="bc")
                    nc.tensor.matmul(bc, ones_sb, rc, start=True, stop=True)
                    bcs = misc.tile([DK, NQS], F32R, tag="bcs")
                    nc.vector.tensor_copy(bcs, bc)
                    nc.vector.tensor_mul(
                        OT_sb[qrow:qrow + DK, tq, :], oa[0:DK, :], bcs)

            # ---- o_proj: Yo[q, do] = O @ Wo^T + bo ------------------------
            yo_all = big.tile([128, NQS // 128, D], F32, tag="yoall")
            for doc in range(4):
                wo = wkp.tile([128, NT, 256], F32R, tag="wk")
                nc.sync.dma_start(
                    out=wo,
                    in_=WoT[:, doc * 256:(doc + 1) * 256].rearrange(
                        "(t p) n -> p t n", p=128),
                )
                for qt in range(NQS // 128):
                    ps = psp.tile([128, 256], F32, tag="ps")
                    for dt in range(NT):
                        nc.tensor.matmul(
                            ps,
                            OT_sb[:, dt, qt * 128:(qt + 1) * 128],
                            wo[:, dt, :],
                            start=(dt == 0),
                            stop=(dt == NT - 1),
                        )
                    nc.vector.tensor_add(
                        yo_all[:, qt, doc * 256:(doc + 1) * 256], ps,
                        bob[:, doc * 256:(doc + 1) * 256])

            # ---- LayerNorm over do, per 128-row q tile --------------------
            for qt in range(NQS // 128):
                row = yo_all[:, qt, :]
                stats = misc.tile([128, 2, 6], F32, tag="stats")
                row2 = row.rearrange("p (s n) -> p s n", s=2)
                for s in range(2):
                    nc.vector.bn_stats(stats[:, s, :], row2[:, s, :])
                mv = misc.tile([128, 2], F32, tag="mv")
                nc.vector.bn_aggr(mv, stats)
                std = misc.tile([128, 1], F32, tag="std")
                nc.scalar.activation(std, mv[:, 1:2], AF.Sqrt, bias=eps_sb)
                rstd = misc.tile([128, 1], F32, tag="rstd")
                nc.vector.reciprocal(rstd, std)
                nc.vector.tensor_scalar(row, row, mv[:, 0:1], rstd,
                                        OP.subtract, OP.mult)
                nc.vector.tensor_mul(row, row, lnwb)
                ob = yop.tile([128, D], F32, tag="ob")
                nc.vector.tensor_add(ob, row, lnbb)
                nc.sync.dma_start(out=out[qt * 128:(qt + 1) * 128, :], in_=ob)
    nc.finalize()
    return nc


# ---------------------------------------------------------------------------
# host wrapper
# ---------------------------------------------------------------------------
_NC_CACHE = {}


def _get_nc():
    if "nc" not in _NC_CACHE:
        _NC_CACHE["nc"] = _build()
    return _NC_CACHE["nc"]


def kernel(q, C, Wq, bq, Wk, bk, Wv, bv, Wo, bo, ln_w, ln_b):
    q = np.ascontiguousarray(np.asarray(q, dtype=np.float32))
    C = np.ascontiguousarray(np.asarray(C, dtype=np.float32))
    f32 = lambda x: np.ascontiguousarray(np.asarray(x, dtype=np.float32))
    WqT = f32(np.asarray(Wq, np.float32).T)
    WkT = f32(np.asarray(Wk, np.float32).T)
    WvT = f32(np.asarray(Wv, np.float32).T)
    WoT = f32(np.asarray(Wo, np.float32).T)
    bq, bk, bv, bo, ln_w, ln_b = map(f32, (bq, bk, bv, bo, ln_w, ln_b))

    CTs = [np.ascontiguousarray(C[b].T) for b in range(B)]
    in_maps = []
    for c in range(8):
        b, qh = c // 2, c % 2
        qTs = np.ascontiguousarray(q[b, qh * NQS:(qh + 1) * NQS, :].T)
        in_maps.append({
            "qT": qTs, "CT": CTs[b],
            "WqT": WqT, "WkT": WkT, "WvT": WvT, "WoT": WoT,
            "bq": bq, "bk": bk, "bv": bv, "bo": bo,
            "lnw": ln_w, "lnb": ln_b,
            "ones64": np.ones(DK, np.float32),
        })

    nc = _get_nc()
    res = run_bass_kernel_spmd(nc, in_maps, core_ids=list(range(8)))

    out = np.empty((B, NQ, D), dtype=np.float32)
    for c in range(8):
        b, qh = c // 2, c % 2
        out[b, qh * NQS:(qh + 1) * NQS, :] = res.results[c]["out"]
    return out
